# revision 9
# baseline (speedup 1.0000x reference)
"""GENConv block (softmax-aggregation message passing + node MLP with 3
training-mode BatchNorms) on 8 Trainium2 NeuronCores.

Strategy
--------
Nodes are sharded contiguously across the 8 cores (12500 nodes each). Every
edge is owned by the core that owns its destination node, so the softmax
segment-reduction is core-local (no cross-core reduce for aggregation).

Host-side preprocessing (index/data movement only, no arithmetic):
  * group each core's edges by destination into windows of 64 nodes,
  * pad every window to a uniform number of 128-edge chunks (SPMD: all 8
    cores run one program, so the schedule must be identical),
  * lay out edge_attr and the gathered x[src] rows in the exact
    [window, partition, chunk*64] tile layout the kernel DMAs,
  * transpose the per-core x slice to [64, nodes] (channel-major).

Device kernel (per core):
  phase 1 (edges):  z = edge_attr + x[src] (DMA-accumulate), m = relu(z),
    ex = exp(m), p = m*ex.  A one-hot matrix M[k, j] = (dst_k == j) built by
    is_equal against an iota row is used to segment-sum via the tensor
    engine: denT += ex^T M, numT += p^T M accumulated in PSUM per window.
    aggrT = numT / denT, zT = aggrT + xT.  (The softmax max-subtraction is
    dropped: messages are in [0, ~9], exp is safely bounded, and the
    reference's 1e-16 guard is far below fp32 epsilon since den >= 1.)
  phase 2 (nodes, channel-major so BN scale/bias are per-partition):
    three matmul+BN+activation stages, each BN needing only a [128, 2]
    AllReduce of (sum, sumsq) over the 8 cores; BN biases b1/b2 cancel
    inside batch-norm and are never used.  Output is PE-transposed back to
    node-major and DMA'd out.
"""
import sys

if "/opt/trn_rl_repo" not in sys.path:
    sys.path.insert(0, "/opt/trn_rl_repo")

import numpy as np
from contextlib import ExitStack

import concourse.bacc as bacc
import concourse.mybir as mybir
import concourse.tile as tile
from concourse.bass_utils import run_bass_kernel_spmd
from concourse.masks import make_identity

F32 = mybir.dt.float32
AX = mybir.AluOpType

N, E, C, CH = 100000, 1600000, 64, 128
NCORES = 8
NPC = N // NCORES          # nodes per core = 12500
WN = 64                    # nodes per window
NW = (NPC + WN - 1) // WN  # windows per core = 196
LAST_WN = NPC - (NW - 1) * WN  # nodes in last window = 20
K = 128                    # edges per chunk (partition dim)
T = 512                    # phase-2 tile width (nodes)
NT = (NPC + T - 1) // T    # 25 tiles, last = 212
EPS_BN = 1e-5
DEN_EPS = 1e-9


def _tile_bounds():
    return [(t * T, min(T, NPC - t * T)) for t in range(NT)]


def build_program(n_fix: int):
    FW = n_fix * C  # free width of an edge tile
    nc = bacc.Bacc(None, target_bir_lowering=False, debug=False)

    ea_d = nc.declare_dram_parameter("ea", [NW, K, FW], F32, isOutput=False)
    xg_d = nc.declare_dram_parameter("xg", [NW, K, FW], F32, isOutput=False)
    wd_d = nc.declare_dram_parameter("wd", [K, NW * n_fix], F32, isOutput=False)
    xt_d = nc.declare_dram_parameter("xt", [C, NPC], F32, isOutput=False)
    w1_d = nc.declare_dram_parameter("w1", [C, CH], F32, isOutput=False)
    w2_d = nc.declare_dram_parameter("w2", [CH, C], F32, isOutput=False)
    wl_d = nc.declare_dram_parameter("wl", [C, C], F32, isOutput=False)
    bn_d = nc.declare_dram_parameter("bn", [CH, 6], F32, isOutput=False)
    y_d = nc.declare_dram_parameter("y", [NPC, C], F32, isOutput=True)

    with tile.TileContext(nc) as tc, ExitStack() as ctx:
        persist = ctx.enter_context(tc.tile_pool(name="persist", bufs=1))
        dram = ctx.enter_context(tc.tile_pool(name="dram", bufs=1, space="DRAM"))

        # ---- persistent tiles -------------------------------------------
        wdst = persist.tile([K, NW * n_fix], F32)
        nc.sync.dma_start(out=wdst[:], in_=wd_d[:, :])
        xT = persist.tile([C, NPC], F32)
        nc.sync.dma_start(out=xT[:], in_=xt_d[:, :])
        w1t = persist.tile([C, CH], F32)
        nc.sync.dma_start(out=w1t[:], in_=w1_d[:, :])
        w2t = persist.tile([CH, C], F32)
        nc.sync.dma_start(out=w2t[:], in_=w2_d[:, :])
        wlt = persist.tile([C, C], F32)
        nc.sync.dma_start(out=wlt[:], in_=wl_d[:, :])
        bnt = persist.tile([CH, 6], F32)
        nc.sync.dma_start(out=bnt[:], in_=bn_d[:, :])

        iota_i = persist.tile([K, C], mybir.dt.int32)
        nc.gpsimd.iota(iota_i[:], pattern=[[1, C]], base=0, channel_multiplier=0)
        iota_f = persist.tile([K, C], F32)
        nc.vector.tensor_copy(out=iota_f[:], in_=iota_i[:])
        ident = persist.tile([K, K], F32)
        make_identity(nc, ident[:])

        zT = persist.tile([C, NPC], F32)
        stats = [persist.tile([CH, 2], F32, name=f"stats{i}") for i in range(3)]
        for s in stats:
            nc.vector.memset(s[:], 0.0)
        eps_t = persist.tile([CH, 1], F32)
        nc.vector.memset(eps_t[:], EPS_BN)

        # ================= phase 1: edges → zT ===========================
        with (
            tc.tile_pool(name="p1z", bufs=3) as p1z,
            tc.tile_pool(name="p1e", bufs=3) as p1e,
            tc.tile_pool(name="p1p", bufs=3) as p1p,
            tc.tile_pool(name="p1m", bufs=3) as p1m,
            tc.tile_pool(name="p1w", bufs=3) as p1w,
            tc.tile_pool(name="psD", bufs=2, space="PSUM") as psD,
            tc.tile_pool(name="psN", bufs=2, space="PSUM") as psN,
        ):
            for w in range(NW):
                nw = WN if w < NW - 1 else LAST_WN
                z = p1z.tile([K, FW], F32, tag="z")
                nc.sync.dma_start(out=z[:], in_=ea_d[w])
                nc.gpsimd.dma_start(out=z[:], in_=xg_d[w], accum_op=AX.add)
                # m = relu(z) in place
                nc.vector.tensor_scalar_max(out=z[:], in0=z[:], scalar1=0.0)
                ex = p1e.tile([K, FW], F32, tag="ex")
                nc.scalar.activation(out=ex[:], in_=z[:],
                                     func=mybir.ActivationFunctionType.Exp)
                pp = p1p.tile([K, FW], F32, tag="pp")
                nc.vector.tensor_tensor(out=pp[:], in0=z[:], in1=ex[:], op=AX.mult)
                mt = p1m.tile([K, FW], F32, tag="mt")
                for n in range(n_fix):
                    nc.vector.tensor_scalar(
                        out=mt[:, n * C:(n + 1) * C], in0=iota_f[:],
                        scalar1=wdst[:, w * n_fix + n: w * n_fix + n + 1],
                        scalar2=None, op0=AX.is_equal)
                pd = psD.tile([C, WN], F32, space="PSUM", tag="pd")
                pn = psN.tile([C, WN], F32, space="PSUM", tag="pn")
                for n in range(n_fix):
                    sl = slice(n * C, (n + 1) * C)
                    nc.tensor.matmul(out=pd[:], lhsT=ex[:, sl], rhs=mt[:, sl],
                                     start=(n == 0), stop=(n == n_fix - 1))
                for n in range(n_fix):
                    sl = slice(n * C, (n + 1) * C)
                    nc.tensor.matmul(out=pn[:], lhsT=pp[:, sl], rhs=mt[:, sl],
                                     start=(n == 0), stop=(n == n_fix - 1))
                den = p1w.tile([C, WN], F32, tag="den")
                nc.vector.tensor_scalar_add(out=den[:], in0=pd[:], scalar1=DEN_EPS)
                rec = p1w.tile([C, WN], F32, tag="rec")
                nc.vector.reciprocal(out=rec[:], in_=den[:])
                agg = p1w.tile([C, WN], F32, tag="agg")
                nc.vector.tensor_tensor(out=agg[:], in0=pn[:], in1=rec[:], op=AX.mult)
                o = w * WN
                nc.vector.tensor_tensor(out=zT[:, o:o + nw], in0=agg[:, :nw],
                                        in1=xT[:, o:o + nw], op=AX.add)

        # ================= phase 2: node MLP =============================
        def bn_coeffs(stats_t, g_col, b_col, rows, sfx):
            """AllReduce [CH,2] stats; return per-channel A, B columns."""
            cc_i = dram.tile([CH, 2], F32, tag=f"cci{sfx}")
            cc_o = dram.tile([CH, 2], F32, addr_space="Shared", tag=f"cco{sfx}")
            nc.sync.dma_start(out=cc_i[:], in_=stats_t[:])
            nc.gpsimd.collective_compute(
                "AllReduce", AX.add, ins=[cc_i[:].opt()], outs=[cc_o[:].opt()],
                replica_groups=[list(range(NCORES))])
            g = persist.tile([CH, 2], F32, tag=f"bnred{sfx}")
            nc.sync.dma_start(out=g[:], in_=cc_o[:])
            r = slice(0, rows)
            mean = persist.tile([CH, 1], F32, tag=f"bnm{sfx}")
            nc.vector.tensor_scalar_mul(out=mean[r], in0=g[r, 0:1], scalar1=1.0 / N)
            msq = persist.tile([CH, 1], F32, tag=f"bnq{sfx}")
            nc.vector.tensor_scalar_mul(out=msq[r], in0=g[r, 1:2], scalar1=1.0 / N)
            var = persist.tile([CH, 1], F32, tag=f"bnv{sfx}")
            nc.vector.tensor_tensor(out=var[r], in0=mean[r], in1=mean[r], op=AX.mult)
            nc.vector.tensor_tensor(out=var[r], in0=msq[r], in1=var[r], op=AX.subtract)
            sd = persist.tile([CH, 1], F32, tag=f"bnsd{sfx}")
            nc.scalar.activation(out=sd[r], in_=var[r],
                                 func=mybir.ActivationFunctionType.Sqrt,
                                 bias=eps_t[r, 0:1])
            rsd = persist.tile([CH, 1], F32, tag=f"bnrs{sfx}")
            nc.vector.reciprocal(out=rsd[r], in_=sd[r])
            A = persist.tile([CH, 1], F32, tag=f"bnA{sfx}")
            nc.vector.tensor_tensor(out=A[r], in0=g_col, in1=rsd[r], op=AX.mult)
            B = persist.tile([CH, 1], F32, tag=f"bnB{sfx}")
            nc.vector.tensor_tensor(out=B[r], in0=mean[r], in1=A[r], op=AX.mult)
            nc.vector.tensor_tensor(out=B[r], in0=b_col, in1=B[r], op=AX.subtract)
            return A, B

        def acc_stats(stats_t, hp, tw, rows, sq_pool, st_pool):
            r = slice(0, rows)
            sq = sq_pool.tile([CH, T], F32, tag="sq")
            sqs = st_pool.tile([CH, 1], F32, tag="sqs")
            nc.scalar.activation(out=sq[r, :tw], in_=hp[r, :tw],
                                 func=mybir.ActivationFunctionType.Square,
                                 accum_out=sqs[r])
            s1 = st_pool.tile([CH, 1], F32, tag="s1")
            nc.vector.reduce_sum(out=s1[r], in_=hp[r, :tw], axis=mybir.AxisListType.X)
            nc.vector.tensor_tensor(out=stats_t[r, 0:1], in0=stats_t[r, 0:1],
                                    in1=s1[r], op=AX.add)
            nc.vector.tensor_tensor(out=stats_t[r, 1:2], in0=stats_t[r, 1:2],
                                    in1=sqs[r], op=AX.add)

        bounds = _tile_bounds()
        with (
            tc.tile_pool(name="p2ps", bufs=2, space="PSUM") as p2ps,
            tc.tile_pool(name="p2ps2", bufs=2, space="PSUM") as p2ps2,
            tc.tile_pool(name="p2ps3", bufs=2, space="PSUM") as p2ps3,
            tc.tile_pool(name="p2sq", bufs=2) as p2sq,
            tc.tile_pool(name="p2st", bufs=4) as p2st,
            tc.tile_pool(name="p2r", bufs=2) as p2r,
            tc.tile_pool(name="pst", bufs=2, space="PSUM") as pst,
            tc.tile_pool(name="p2o", bufs=2) as p2o,
        ):
            # -- 2a: h1 = zT'W1 stats ------------------------------------
            for o, tw in bounds:
                h1p = p2ps.tile([CH, T], F32, space="PSUM", tag="h1p")
                nc.tensor.matmul(out=h1p[:, :tw], lhsT=w1t[:], rhs=zT[:, o:o + tw],
                                 start=True, stop=True)
                acc_stats(stats[0], h1p, tw, CH, p2sq, p2st)
            A1, B1 = bn_coeffs(stats[0], bnt[:, 0:1], bnt[:, 1:2], CH, 1)

            def chain_h2(o, tw):
                """h2p PSUM tile for node range [o, o+tw) (recomputed)."""
                h1p = p2ps.tile([CH, T], F32, space="PSUM", tag="h1p")
                nc.tensor.matmul(out=h1p[:, :tw], lhsT=w1t[:], rhs=zT[:, o:o + tw],
                                 start=True, stop=True)
                r1 = p2r.tile([CH, T], F32, tag="r1")
                nc.scalar.activation(out=r1[:, :tw], in_=h1p[:, :tw],
                                     func=mybir.ActivationFunctionType.Relu,
                                     scale=A1[:, 0:1], bias=B1[:, 0:1])
                h2p = p2ps2.tile([C, T], F32, space="PSUM", tag="h2p")
                nc.tensor.matmul(out=h2p[:, :tw], lhsT=w2t[:], rhs=r1[:, :tw],
                                 start=True, stop=True)
                return h2p

            def chain_h3(o, tw):
                h2p = chain_h2(o, tw)
                u = p2r.tile([C, T], F32, tag="u")
                nc.scalar.activation(out=u[:, :tw], in_=h2p[:, :tw],
                                     func=mybir.ActivationFunctionType.Silu,
                                     scale=A2[0:C, 0:1], bias=B2[0:C, 0:1])
                h3p = p2ps3.tile([C, T], F32, space="PSUM", tag="h3p")
                nc.tensor.matmul(out=h3p[:, :tw], lhsT=wlt[:], rhs=u[:, :tw],
                                 start=True, stop=True)
                return h3p

            # -- 2b: h2 = relu(bn1(h1))'W2, stats -------------------------
            for o, tw in bounds:
                h2p = chain_h2(o, tw)
                acc_stats(stats[1], h2p, tw, C, p2sq, p2st)
            A2, B2 = bn_coeffs(stats[1], bnt[0:C, 2:3], bnt[0:C, 3:4], C, 2)

            # -- 2c: u = silu(bn2(h2)), h3 = u'Wl, stats ------------------
            for o, tw in bounds:
                h3p = chain_h3(o, tw)
                acc_stats(stats[2], h3p, tw, C, p2sq, p2st)
            A3, B3 = bn_coeffs(stats[2], bnt[0:C, 4:5], bnt[0:C, 5:6], C, 3)

            # -- 2d: y = silu(bn3(h3)), transpose, store ------------------
            for o, tw in bounds:
                h3p = chain_h3(o, tw)
                ot = p2o.tile([C, T], F32, tag="ot")
                nc.scalar.activation(out=ot[:, :tw], in_=h3p[:, :tw],
                                     func=mybir.ActivationFunctionType.Silu,
                                     scale=A3[0:C, 0:1], bias=B3[0:C, 0:1])
                nblk = (tw + K - 1) // K
                yt = p2o.tile([K, nblk * C], F32, tag="yt")
                for j in range(nblk):
                    bw = min(K, tw - j * K)
                    tp = pst.tile([K, C], F32, space="PSUM", tag="tp")
                    nc.tensor.transpose(out=tp[:bw, :], in_=ot[:, j * K:j * K + bw],
                                        identity=ident[0:C, 0:C])
                    nc.scalar.activation(out=yt[:bw, j * C:(j + 1) * C],
                                         in_=tp[:bw, :],
                                         func=mybir.ActivationFunctionType.Copy)
                for j in range(nblk):
                    bw = min(K, tw - j * K)
                    nc.sync.dma_start(out=y_d[o + j * K: o + j * K + bw, :],
                                      in_=yt[:bw, j * C:(j + 1) * C])
    nc.finalize()
    return nc


def preprocess(x, edge_index, edge_attr, W1, W2, Wl, g_mlp, be_mlp, g1, be1,
               g2, be2):
    src = np.asarray(edge_index[0])
    dst = np.asarray(edge_index[1])
    x = np.asarray(x, dtype=np.float32)
    edge_attr = np.asarray(edge_attr, dtype=np.float32)

    core = dst // NPC
    local = dst - core * NPC
    win = local // WN
    gwin = core * NW + win
    order = np.argsort(gwin, kind="stable")
    gw_s = gwin[order]
    counts = np.bincount(gwin, minlength=NCORES * NW)
    n_fix = int(np.ceil(counts.max() / K))
    starts = np.zeros(NCORES * NW, np.int64)
    np.cumsum(counts[:-1], out=starts[1:])
    rank = np.arange(E, dtype=np.int64) - starts[gw_s]

    e_core = core[order]
    e_w = win[order]
    e_n = (rank // K).astype(np.int64)
    e_p = (rank % K).astype(np.int64)

    ea_w = np.zeros((NCORES, NW, K, n_fix, C), np.float32)
    xg_w = np.zeros((NCORES, NW, K, n_fix, C), np.float32)
    wd_w = np.full((NCORES, NW, K, n_fix), 255.0, np.float32)
    ea_w[e_core, e_w, e_p, e_n] = edge_attr[order]
    xg_w[e_core, e_w, e_p, e_n] = x[src[order]]
    wd_w[e_core, e_w, e_p, e_n] = (local[order] - e_w * WN).astype(np.float32)
    # wdst preload layout: [K, NW*n_fix]
    wd_w = np.ascontiguousarray(wd_w.transpose(0, 2, 1, 3).reshape(NCORES, K, NW * n_fix))
    ea_w = ea_w.reshape(NCORES, NW, K, n_fix * C)
    xg_w = xg_w.reshape(NCORES, NW, K, n_fix * C)

    bn = np.zeros((CH, 6), np.float32)
    bn[:, 0] = g_mlp
    bn[:, 1] = be_mlp
    bn[:C, 2] = g1
    bn[:C, 3] = be1
    bn[:C, 4] = g2
    bn[:C, 5] = be2

    in_maps = []
    for c in range(NCORES):
        xs = np.ascontiguousarray(x[c * NPC:(c + 1) * NPC].T)
        in_maps.append(dict(
            ea=ea_w[c], xg=xg_w[c], wd=wd_w[c], xt=xs,
            w1=np.asarray(W1, np.float32), w2=np.asarray(W2, np.float32),
            wl=np.asarray(Wl, np.float32), bn=bn,
        ))
    return in_maps, n_fix


_PROG_CACHE = {}


def kernel(x, edge_index, edge_attr, pos, W1, b1, g_mlp, be_mlp, W2, b2,
           g1, be1, Wl, g2, be2):
    # b1/b2 cancel inside the batch norms that directly follow them; pos is
    # unused by the reference.
    in_maps, n_fix = preprocess(x, edge_index, edge_attr, W1, W2, Wl,
                                g_mlp, be_mlp, g1, be1, g2, be2)
    if n_fix not in _PROG_CACHE:
        _PROG_CACHE[n_fix] = build_program(n_fix)
    nc = _PROG_CACHE[n_fix]
    r = run_bass_kernel_spmd(nc, in_maps, list(range(NCORES)))
    return np.concatenate([r.results[c]["y"] for c in range(NCORES)], axis=0)


# revision 23
# speedup vs baseline: 141.1958x; 141.1958x over previous
"""GENConv block (softmax-aggregation message passing + node MLP with 3
training-mode BatchNorms) on 8 Trainium2 NeuronCores.

Strategy
--------
Nodes are sharded contiguously across the 8 cores (12500 nodes each). Every
edge is owned by the core that owns its destination node, so the softmax
segment-reduction is core-local (no cross-core reduce for aggregation).

Host-side preprocessing (index/data movement only, no arithmetic):
  * group each core's edges by destination into windows of 64 nodes,
  * pad every window to a uniform number of 128-edge chunks (SPMD: all 8
    cores run one program, so the schedule must be identical),
  * lay out edge_attr and the gathered x[src] rows in the exact
    [window, partition, chunk*64] tile layout the kernel DMAs,
  * transpose the per-core x slice to [64, nodes] (channel-major).

Device kernel (per core):
  phase 1 (edges):  z = edge_attr + x[src] (DMA-accumulate), m = relu(z),
    ex = exp(m), p = m*ex.  A one-hot matrix M[k, j] = (dst_k == j) built by
    is_equal against an iota row is used to segment-sum via the tensor
    engine: denT += ex^T M, numT += p^T M accumulated in PSUM per window.
    aggrT = numT / denT, zT = aggrT + xT.  (The softmax max-subtraction is
    dropped: messages are in [0, ~9], exp is safely bounded, and the
    reference's 1e-16 guard is far below fp32 epsilon since den >= 1.)
  phase 2 (nodes, channel-major so BN scale/bias are per-partition):
    three matmul+BN+activation stages, each BN needing only a [128, 2]
    AllReduce of (sum, sumsq) over the 8 cores; BN biases b1/b2 cancel
    inside batch-norm and are never used.  Output is PE-transposed back to
    node-major and DMA'd out.
"""
import sys

if "/opt/trn_rl_repo" not in sys.path:
    sys.path.insert(0, "/opt/trn_rl_repo")

import numpy as np
from contextlib import ExitStack

import concourse.bacc as bacc
import concourse.mybir as mybir
import concourse.tile as tile
from concourse.bass_utils import run_bass_kernel_spmd
from concourse.masks import make_identity

F32 = mybir.dt.float32
F16 = mybir.dt.float16
F8 = mybir.dt.float8e4
AX = mybir.AluOpType
LN4 = 1.3862943611198906

N, E, C, CH = 100000, 1600000, 64, 128
NCORES = 8
NPC = N // NCORES          # nodes per core = 12500
WN = 64                    # nodes per window
NW = (NPC + WN - 1) // WN  # windows per core = 196
LAST_WN = NPC - (NW - 1) * WN  # nodes in last window = 20
K = 128                    # edges per chunk (partition dim)
T = 512                    # phase-2 tile width (nodes)
NT = (NPC + T - 1) // T    # 25 tiles, last = 212
EPS_BN = 1e-5
DEN_EPS = 2.5e-10


def _tile_bounds():
    return [(t * T, min(T, NPC - t * T)) for t in range(NT)]


def build_program(n_fix: int):
    FW = n_fix * C  # free width of an edge tile
    nc = bacc.Bacc(None, target_bir_lowering=False, debug=False)

    ea_d = nc.declare_dram_parameter("ea", [NW, K, FW], F32, isOutput=False)
    xg_d = nc.declare_dram_parameter("xg", [NW, K, FW], F32, isOutput=False)
    m_d = nc.declare_dram_parameter("mh", [NW, K, FW], F8, isOutput=False)
    xt_d = nc.declare_dram_parameter("xt", [C, NPC], F32, isOutput=False)
    w1_d = nc.declare_dram_parameter("w1", [C, CH], F16, isOutput=False)
    w2_d = nc.declare_dram_parameter("w2", [CH, C], F16, isOutput=False)
    wl_d = nc.declare_dram_parameter("wl", [C, C], F16, isOutput=False)
    bn_d = nc.declare_dram_parameter("bn", [CH, 6], F32, isOutput=False)
    y_d = nc.declare_dram_parameter("y", [NPC, C], F32, isOutput=True)

    with tile.TileContext(nc) as tc, ExitStack() as ctx:
        persist = ctx.enter_context(tc.tile_pool(name="persist", bufs=1))
        dram = ctx.enter_context(tc.tile_pool(name="dram", bufs=1, space="DRAM"))

        # ---- persistent tiles -------------------------------------------
        xT = persist.tile([C, NPC], F32)
        nc.sync.dma_start(out=xT[:], in_=xt_d[:, :])
        w1t = persist.tile([C, CH], F16)
        nc.sync.dma_start(out=w1t[:], in_=w1_d[:, :])
        w2t = persist.tile([CH, C], F16)
        nc.sync.dma_start(out=w2t[:], in_=w2_d[:, :])
        wlt = persist.tile([C, C], F16)
        nc.sync.dma_start(out=wlt[:], in_=wl_d[:, :])
        bnt = persist.tile([CH, 6], F32)
        nc.sync.dma_start(out=bnt[:], in_=bn_d[:, :])

        ident16 = persist.tile([K, K], F16)
        make_identity(nc, ident16[:])

        bounds0 = _tile_bounds()
        zTs = [persist.tile([C, tw], F16, name=f"zT{i}") for i, (_, tw) in enumerate(bounds0)]
        r1T = persist.tile([CH, NPC], F16)
        uT = persist.tile([C, NPC], F16)
        stats = [persist.tile([CH, 2], F32, name=f"stats{i}") for i in range(3)]
        for s in stats:
            nc.vector.memset(s[:], 0.0)
        eps_t = persist.tile([CH, 1], F32)
        nc.vector.memset(eps_t[:], EPS_BN)
        nln4_t = persist.tile([CH, 1], F32)
        nc.vector.memset(nln4_t[:], -LN4)
        eps_den = persist.tile([CH, 1], F32)
        nc.vector.memset(eps_den[:], DEN_EPS)

        # ================= phase 1: edges → zT ===========================
        with (
            tc.tile_pool(name="p1z", bufs=4) as p1z,
            tc.tile_pool(name="p1e", bufs=4) as p1e,
            tc.tile_pool(name="p1p", bufs=4) as p1p,
            tc.tile_pool(name="p1m", bufs=4) as p1m,
            tc.tile_pool(name="p1w", bufs=4) as p1w,
            tc.tile_pool(name="psD", bufs=4, space="PSUM") as psD,
            tc.tile_pool(name="psN", bufs=4, space="PSUM") as psN,
        ):
            assert NW % 2 == 0
            for wp in range(NW // 2):
                w0 = 2 * wp
                z = p1z.tile([K, 2 * FW], F32, tag="z")
                nc.sync.dma_start(
                    out=z[:].rearrange("p (t f) -> p t f", t=2),
                    in_=ea_d[w0:w0 + 2].rearrange("t p f -> p t f"))
                nc.gpsimd.dma_start(
                    out=z[:].rearrange("p (t f) -> p t f", t=2),
                    in_=xg_d[w0:w0 + 2].rearrange("t p f -> p t f"),
                    accum_op=AX.add)
                mt = p1m.tile([K, 2 * FW], F8, tag="mt")
                mt_eng = nc.gpsimd if wp % 2 == 0 else nc.sync
                mt_eng.dma_start(
                    out=mt[:].rearrange("p (t f) -> p t f", t=2),
                    in_=m_d[w0:w0 + 2].rearrange("t p f -> p t f"))
                # ex = exp(relu(z))/4 == max(exp(z)/4, 1/4) and
                # p  = relu(z)*exp(relu(z))/4 == max(z*ex, 0): both identities
                # avoid materializing relu(z).  The /4 (exp bias -ln4) keeps p
                # well inside fp16 range; the num/den ratio is unchanged.
                ex = p1e.tile([K, 2 * FW], F16, tag="ex")
                nc.scalar.activation(out=ex[:], in_=z[:],
                                     func=mybir.ActivationFunctionType.Exp,
                                     bias=nln4_t[:, 0:1])
                nc.vector.tensor_scalar_max(out=ex[:], in0=ex[:], scalar1=0.25)
                pp = p1p.tile([K, 2 * FW], F16, tag="pp")
                nc.vector.tensor_tensor(out=pp[:], in0=z[:], in1=ex[:], op=AX.mult)
                nc.vector.tensor_scalar_max(out=pp[:], in0=pp[:], scalar1=0.0)
                dens = p1w.tile([C, 2 * WN], F32, tag="dens")
                pns = []
                for t in range(2):
                    base = t * FW
                    pd = psD.tile([C, WN], F32, space="PSUM", tag="pd")
                    pn = psN.tile([C, WN], F32, space="PSUM", tag="pn")
                    for n in range(n_fix):
                        sl = slice(base + n * C, base + (n + 1) * C)
                        nc.tensor.matmul(out=pd[:], lhsT=ex[:, sl], rhs=mt[:, sl],
                                         start=(n == 0), stop=(n == n_fix - 1))
                    for n in range(n_fix):
                        sl = slice(base + n * C, base + (n + 1) * C)
                        nc.tensor.matmul(out=pn[:], lhsT=pp[:, sl], rhs=mt[:, sl],
                                         start=(n == 0), stop=(n == n_fix - 1))
                    nc.scalar.activation(out=dens[:, t * WN:(t + 1) * WN], in_=pd[:],
                                         func=mybir.ActivationFunctionType.Identity,
                                         bias=eps_den[0:C, 0:1])
                    pns.append(pn)
                rec = p1w.tile([C, 2 * WN], F32, tag="rec")
                nc.vector.reciprocal(out=rec[:], in_=dens[:])
                agg = p1w.tile([C, 2 * WN], F32, tag="agg")
                for t in range(2):
                    cs = slice(t * WN, (t + 1) * WN)
                    nc.vector.tensor_tensor(out=agg[:, cs], in0=pns[t][:],
                                            in1=rec[:, cs], op=AX.mult)
                npair = 2 * WN if wp < NW // 2 - 1 else WN + LAST_WN
                o = w0 * WN
                ti, to = divmod(o, T)
                nc.vector.tensor_tensor(out=zTs[ti][:, to:to + npair],
                                        in0=agg[:, :npair],
                                        in1=xT[:, o:o + npair], op=AX.add)

        # ================= phase 2: node MLP =============================
        def bn_coeffs(stats_t, g_col, b_col, rows, sfx):
            """AllReduce [CH,2] stats; return per-channel A, B columns."""
            cc_i = dram.tile([CH, 2], F32, tag=f"cci{sfx}")
            cc_o = dram.tile([CH, 2], F32, addr_space="Shared", tag=f"cco{sfx}")
            nc.sync.dma_start(out=cc_i[:], in_=stats_t[:])
            nc.gpsimd.collective_compute(
                "AllReduce", AX.add, ins=[cc_i[:].opt()], outs=[cc_o[:].opt()],
                replica_groups=[list(range(NCORES))])
            g = persist.tile([CH, 2], F32, tag=f"bnred{sfx}")
            nc.sync.dma_start(out=g[:], in_=cc_o[:])
            r = slice(0, rows)
            mean = persist.tile([CH, 1], F32, tag=f"bnm{sfx}")
            nc.vector.tensor_scalar_mul(out=mean[r], in0=g[r, 0:1], scalar1=1.0 / N)
            msq = persist.tile([CH, 1], F32, tag=f"bnq{sfx}")
            nc.vector.tensor_scalar_mul(out=msq[r], in0=g[r, 1:2], scalar1=1.0 / N)
            var = persist.tile([CH, 1], F32, tag=f"bnv{sfx}")
            nc.vector.tensor_tensor(out=var[r], in0=mean[r], in1=mean[r], op=AX.mult)
            nc.vector.tensor_tensor(out=var[r], in0=msq[r], in1=var[r], op=AX.subtract)
            sd = persist.tile([CH, 1], F32, tag=f"bnsd{sfx}")
            nc.scalar.activation(out=sd[r], in_=var[r],
                                 func=mybir.ActivationFunctionType.Sqrt,
                                 bias=eps_t[r, 0:1])
            rsd = persist.tile([CH, 1], F32, tag=f"bnrs{sfx}")
            nc.vector.reciprocal(out=rsd[r], in_=sd[r])
            A = persist.tile([CH, 1], F32, tag=f"bnA{sfx}")
            nc.vector.tensor_tensor(out=A[r], in0=g_col, in1=rsd[r], op=AX.mult)
            B = persist.tile([CH, 1], F32, tag=f"bnB{sfx}")
            nc.vector.tensor_tensor(out=B[r], in0=mean[r], in1=A[r], op=AX.mult)
            nc.vector.tensor_tensor(out=B[r], in0=b_col, in1=B[r], op=AX.subtract)
            return A, B

        def acc_stats(stats_t, hp, tw, rows, sq_pool, st_pool):
            r = slice(0, rows)
            sq = sq_pool.tile([CH, T], F32, tag="sq")
            sqs = st_pool.tile([CH, 1], F32, tag="sqs")
            nc.scalar.activation(out=sq[r, :tw], in_=hp[r, :tw],
                                 func=mybir.ActivationFunctionType.Square,
                                 accum_out=sqs[r])
            s1 = st_pool.tile([CH, 1], F32, tag="s1")
            nc.vector.reduce_sum(out=s1[r], in_=hp[r, :tw], axis=mybir.AxisListType.X)
            nc.vector.tensor_tensor(out=stats_t[r, 0:1], in0=stats_t[r, 0:1],
                                    in1=s1[r], op=AX.add)
            nc.vector.tensor_tensor(out=stats_t[r, 1:2], in0=stats_t[r, 1:2],
                                    in1=sqs[r], op=AX.add)

        bounds = _tile_bounds()
        with (
            tc.tile_pool(name="p2ps", bufs=2, space="PSUM") as p2ps,
            tc.tile_pool(name="p2ps2", bufs=2, space="PSUM") as p2ps2,
            tc.tile_pool(name="p2ps3", bufs=2, space="PSUM") as p2ps3,
            tc.tile_pool(name="p2sq", bufs=2) as p2sq,
            tc.tile_pool(name="p2st", bufs=4) as p2st,
            tc.tile_pool(name="p2r", bufs=2) as p2r,
            tc.tile_pool(name="pst", bufs=2, space="PSUM") as pst,
            tc.tile_pool(name="p2o", bufs=2) as p2o,
        ):
            # -- 2a: h1 = zT'W1 stats ------------------------------------
            for ti, (o, tw) in enumerate(bounds):
                h1p = p2ps.tile([CH, T], F32, space="PSUM", tag="h1p")
                nc.tensor.matmul(out=h1p[:, :tw], lhsT=w1t[:], rhs=zTs[ti][:, :tw],
                                 start=True, stop=True)
                acc_stats(stats[0], h1p, tw, CH, p2sq, p2st)
            A1, B1 = bn_coeffs(stats[0], bnt[:, 0:1], bnt[:, 1:2], CH, 1)

            # -- 2b: r1 = relu(bn1(h1)) persisted; h2 stats ---------------
            for ti, (o, tw) in enumerate(bounds):
                h1p = p2ps.tile([CH, T], F32, space="PSUM", tag="h1p")
                nc.tensor.matmul(out=h1p[:, :tw], lhsT=w1t[:], rhs=zTs[ti][:, :tw],
                                 start=True, stop=True)
                nc.scalar.activation(out=r1T[:, o:o + tw], in_=h1p[:, :tw],
                                     func=mybir.ActivationFunctionType.Relu,
                                     scale=A1[:, 0:1], bias=B1[:, 0:1])
                h2p = p2ps2.tile([C, T], F32, space="PSUM", tag="h2p")
                nc.tensor.matmul(out=h2p[:, :tw], lhsT=w2t[:], rhs=r1T[:, o:o + tw],
                                 start=True, stop=True)
                acc_stats(stats[1], h2p, tw, C, p2sq, p2st)
            A2, B2 = bn_coeffs(stats[1], bnt[0:C, 2:3], bnt[0:C, 3:4], C, 2)

            # -- 2c: u = silu(bn2(h2)) persisted; h3 stats ----------------
            for o, tw in bounds:
                h2p = p2ps2.tile([C, T], F32, space="PSUM", tag="h2p")
                nc.tensor.matmul(out=h2p[:, :tw], lhsT=w2t[:], rhs=r1T[:, o:o + tw],
                                 start=True, stop=True)
                nc.scalar.activation(out=uT[:, o:o + tw], in_=h2p[:, :tw],
                                     func=mybir.ActivationFunctionType.Silu,
                                     scale=A2[0:C, 0:1], bias=B2[0:C, 0:1])
                h3p = p2ps3.tile([C, T], F32, space="PSUM", tag="h3p")
                nc.tensor.matmul(out=h3p[:, :tw], lhsT=wlt[:], rhs=uT[:, o:o + tw],
                                 start=True, stop=True)
                acc_stats(stats[2], h3p, tw, C, p2sq, p2st)
            A3, B3 = bn_coeffs(stats[2], bnt[0:C, 4:5], bnt[0:C, 5:6], C, 3)

            # -- 2d: y = silu(bn3(h3)), transpose, store ------------------
            for o, tw in bounds:
                h3p = p2ps3.tile([C, T], F32, space="PSUM", tag="h3p")
                nc.tensor.matmul(out=h3p[:, :tw], lhsT=wlt[:], rhs=uT[:, o:o + tw],
                                 start=True, stop=True)
                ot = p2o.tile([C, T], F16, tag="ot")
                nc.scalar.activation(out=ot[:, :tw], in_=h3p[:, :tw],
                                     func=mybir.ActivationFunctionType.Silu,
                                     scale=A3[0:C, 0:1], bias=B3[0:C, 0:1])
                nblk = (tw + K - 1) // K
                yt = p2o.tile([K, nblk * C], F32, tag="yt")
                tp = pst.tile([K, 4 * C], F16, space="PSUM", tag="tp")
                for j in range(nblk):
                    bw = min(K, tw - j * K)
                    nc.tensor.transpose(out=tp[:bw, j * C:(j + 1) * C],
                                        in_=ot[:, j * K:j * K + bw],
                                        identity=ident16[0:C, 0:C])
                if tw == T:
                    nc.scalar.activation(out=yt[:], in_=tp[:, :nblk * C],
                                         func=mybir.ActivationFunctionType.Copy)
                else:
                    for j in range(nblk):
                        bw = min(K, tw - j * K)
                        nc.scalar.activation(out=yt[:bw, j * C:(j + 1) * C],
                                             in_=tp[:bw, j * C:(j + 1) * C],
                                             func=mybir.ActivationFunctionType.Copy)
                for j in range(nblk):
                    bw = min(K, tw - j * K)
                    nc.sync.dma_start(out=y_d[o + j * K: o + j * K + bw, :],
                                      in_=yt[:bw, j * C:(j + 1) * C])
    nc.finalize()
    return nc


def preprocess(x, edge_index, edge_attr, W1, W2, Wl, g_mlp, be_mlp, g1, be1,
               g2, be2):
    src = np.asarray(edge_index[0])
    dst = np.asarray(edge_index[1])
    x = np.asarray(x, dtype=np.float32)
    edge_attr = np.asarray(edge_attr, dtype=np.float32)

    core = dst // NPC
    local = dst - core * NPC
    win = local // WN
    gwin = core * NW + win
    order = np.argsort(gwin, kind="stable")
    gw_s = gwin[order]
    counts = np.bincount(gwin, minlength=NCORES * NW)
    n_fix = int(np.ceil(counts.max() / K))
    starts = np.zeros(NCORES * NW, np.int64)
    np.cumsum(counts[:-1], out=starts[1:])
    rank = np.arange(E, dtype=np.int64) - starts[gw_s]

    e_core = core[order]
    e_w = win[order]
    e_n = (rank // K).astype(np.int64)
    e_p = (rank % K).astype(np.int64)

    ea_w = np.zeros((NCORES, NW, K, n_fix, C), np.float32)
    xg_w = np.zeros((NCORES, NW, K, n_fix, C), np.float32)
    wd_w = np.full((NCORES, NW, K, n_fix), 255.0, np.float32)
    ea_w[e_core, e_w, e_p, e_n] = edge_attr[order]
    xg_w[e_core, e_w, e_p, e_n] = x[src[order]]
    wd_w[e_core, e_w, e_p, e_n] = (local[order] - e_w * WN).astype(np.float32)
    # wdst preload layout: [K, NW*n_fix]
    import ml_dtypes
    mh_w = (wd_w[..., None] == np.arange(C, dtype=np.float32)).astype(ml_dtypes.float8_e4m3)
    mh_w = mh_w.reshape(NCORES, NW, K, n_fix * C)
    ea_w = ea_w.reshape(NCORES, NW, K, n_fix * C)
    xg_w = xg_w.reshape(NCORES, NW, K, n_fix * C)

    bn = np.zeros((CH, 6), np.float32)
    bn[:, 0] = g_mlp
    bn[:, 1] = be_mlp
    bn[:C, 2] = g1
    bn[:C, 3] = be1
    bn[:C, 4] = g2
    bn[:C, 5] = be2

    in_maps = []
    for c in range(NCORES):
        xs = np.ascontiguousarray(x[c * NPC:(c + 1) * NPC].T)
        in_maps.append(dict(
            ea=ea_w[c], xg=xg_w[c], mh=mh_w[c], xt=xs,
            w1=np.asarray(W1, np.float16), w2=np.asarray(W2, np.float16),
            wl=np.asarray(Wl, np.float16), bn=bn,
        ))
    return in_maps, n_fix


_PROG_CACHE = {}


def kernel(x, edge_index, edge_attr, pos, W1, b1, g_mlp, be_mlp, W2, b2,
           g1, be1, Wl, g2, be2):
    # b1/b2 cancel inside the batch norms that directly follow them; pos is
    # unused by the reference.
    in_maps, n_fix = preprocess(x, edge_index, edge_attr, W1, W2, Wl,
                                g_mlp, be_mlp, g1, be1, g2, be2)
    if n_fix not in _PROG_CACHE:
        _PROG_CACHE[n_fix] = build_program(n_fix)
    nc = _PROG_CACHE[n_fix]
    r = run_bass_kernel_spmd(nc, in_maps, list(range(NCORES)))
    return np.concatenate([r.results[c]["y"] for c in range(NCORES)], axis=0)


# revision 26
# speedup vs baseline: 144.8294x; 1.0257x over previous
"""GENConv block (softmax-aggregation message passing + node MLP with 3
training-mode BatchNorms) on 8 Trainium2 NeuronCores.

Strategy
--------
Nodes are sharded contiguously across the 8 cores (12500 nodes each). Every
edge is owned by the core that owns its destination node, so the softmax
segment-reduction is core-local (no cross-core reduce for aggregation).

Host-side preprocessing (index/data movement only, no arithmetic):
  * group each core's edges by destination into windows of 64 nodes,
  * pad every window to a uniform number of 128-edge chunks (SPMD: all 8
    cores run one program, so the schedule must be identical),
  * lay out edge_attr and the gathered x[src] rows in the exact
    [window, partition, chunk*64] tile layout the kernel DMAs,
  * transpose the per-core x slice to [64, nodes] (channel-major).

Device kernel (per core):
  phase 1 (edges):  z = edge_attr + x[src] (DMA-accumulate), m = relu(z),
    ex = exp(m), p = m*ex.  A one-hot matrix M[k, j] = (dst_k == j) built by
    is_equal against an iota row is used to segment-sum via the tensor
    engine: denT += ex^T M, numT += p^T M accumulated in PSUM per window.
    aggrT = numT / denT, zT = aggrT + xT.  (The softmax max-subtraction is
    dropped: messages are in [0, ~9], exp is safely bounded, and the
    reference's 1e-16 guard is far below fp32 epsilon since den >= 1.)
  phase 2 (nodes, channel-major so BN scale/bias are per-partition):
    three matmul+BN+activation stages, each BN needing only a [128, 2]
    AllReduce of (sum, sumsq) over the 8 cores; BN biases b1/b2 cancel
    inside batch-norm and are never used.  Output is PE-transposed back to
    node-major and DMA'd out.
"""
import sys

if "/opt/trn_rl_repo" not in sys.path:
    sys.path.insert(0, "/opt/trn_rl_repo")

import numpy as np
from contextlib import ExitStack

import concourse.bacc as bacc
import concourse.mybir as mybir
import concourse.tile as tile
from concourse.bass_utils import run_bass_kernel_spmd
from concourse.masks import make_identity

F32 = mybir.dt.float32
F16 = mybir.dt.float16
F8 = mybir.dt.float8e4
AX = mybir.AluOpType
LN4 = 1.3862943611198906

N, E, C, CH = 100000, 1600000, 64, 128
NCORES = 8
NPC = N // NCORES          # nodes per core = 12500
WN = 64                    # nodes per window
NW = (NPC + WN - 1) // WN  # windows per core = 196
LAST_WN = NPC - (NW - 1) * WN  # nodes in last window = 20
K = 128                    # edges per chunk (partition dim)
T = 512                    # phase-2 tile width (nodes)
NT = (NPC + T - 1) // T    # 25 tiles, last = 212
EPS_BN = 1e-5
DEN_EPS = 2.5e-10


def _tile_bounds():
    return [(t * T, min(T, NPC - t * T)) for t in range(NT)]


def build_program(n_fix: int):
    FW = n_fix * C  # free width of an edge tile
    nc = bacc.Bacc(None, target_bir_lowering=False, debug=False)

    ea_d = nc.declare_dram_parameter("ea", [NW, K, FW], F32, isOutput=False)
    xg_d = nc.declare_dram_parameter("xg", [NW, K, FW], F32, isOutput=False)
    m_d = nc.declare_dram_parameter("mh", [NW, K, FW], F8, isOutput=False)
    xt_d = nc.declare_dram_parameter("xt", [C, NPC], F32, isOutput=False)
    w1_d = nc.declare_dram_parameter("w1", [C, CH], F16, isOutput=False)
    w2_d = nc.declare_dram_parameter("w2", [CH, C], F16, isOutput=False)
    wl_d = nc.declare_dram_parameter("wl", [C, C], F16, isOutput=False)
    bn_d = nc.declare_dram_parameter("bn", [CH, 6], F32, isOutput=False)
    y_d = nc.declare_dram_parameter("y", [NPC, C], F32, isOutput=True)

    with tile.TileContext(nc) as tc, ExitStack() as ctx:
        persist = ctx.enter_context(tc.tile_pool(name="persist", bufs=1))
        dram = ctx.enter_context(tc.tile_pool(name="dram", bufs=1, space="DRAM"))

        # ---- persistent tiles -------------------------------------------
        xT = persist.tile([C, NPC], F32)
        nc.sync.dma_start(out=xT[:], in_=xt_d[:, :])
        w1t = persist.tile([C, CH], F16)
        nc.sync.dma_start(out=w1t[:], in_=w1_d[:, :])
        w2t = persist.tile([CH, C], F16)
        nc.sync.dma_start(out=w2t[:], in_=w2_d[:, :])
        wlt = persist.tile([C, C], F16)
        nc.sync.dma_start(out=wlt[:], in_=wl_d[:, :])
        bnt = persist.tile([CH, 6], F32)
        nc.sync.dma_start(out=bnt[:], in_=bn_d[:, :])

        ident16 = persist.tile([K, K], F16)
        make_identity(nc, ident16[:])

        bounds0 = _tile_bounds()
        zTs = [persist.tile([C, tw], F16, name=f"zT{i}") for i, (_, tw) in enumerate(bounds0)]
        r1T = persist.tile([CH, NPC], F16)
        uT = persist.tile([C, NPC], F16)
        stats = [persist.tile([CH, 2], F32, name=f"stats{i}") for i in range(3)]
        for s in stats:
            nc.vector.memset(s[:], 0.0)
        eps_t = persist.tile([CH, 1], F32)
        nc.vector.memset(eps_t[:], EPS_BN)
        nln4_t = persist.tile([CH, 1], F32)
        nc.vector.memset(nln4_t[:], -LN4)
        eps_den = persist.tile([CH, 1], F32)
        nc.vector.memset(eps_den[:], DEN_EPS)

        # ================= phase 1: edges → zT ===========================
        with (
            tc.tile_pool(name="p1z", bufs=4) as p1z,
            tc.tile_pool(name="p1e", bufs=4) as p1e,
            tc.tile_pool(name="p1p", bufs=4) as p1p,
            tc.tile_pool(name="p1m", bufs=4) as p1m,
            tc.tile_pool(name="p1w", bufs=4) as p1w,
            tc.tile_pool(name="psD", bufs=4, space="PSUM") as psD,
            tc.tile_pool(name="psN", bufs=4, space="PSUM") as psN,
        ):
            assert NW % 2 == 0
            for wp in range(NW // 2):
                w0 = 2 * wp
                z = p1z.tile([K, 2 * FW], F32, tag="z")
                nc.sync.dma_start(
                    out=z[:].rearrange("p (t f) -> p t f", t=2),
                    in_=ea_d[w0:w0 + 2].rearrange("t p f -> p t f"))
                nc.gpsimd.dma_start(
                    out=z[:].rearrange("p (t f) -> p t f", t=2),
                    in_=xg_d[w0:w0 + 2].rearrange("t p f -> p t f"),
                    accum_op=AX.add)
                mt = p1m.tile([K, 2 * FW], F8, tag="mt")
                mt_eng = nc.gpsimd if wp % 2 == 0 else nc.sync
                mt_eng.dma_start(
                    out=mt[:].rearrange("p (t f) -> p t f", t=2),
                    in_=m_d[w0:w0 + 2].rearrange("t p f -> p t f"))
                # ex = exp(relu(z))/4 == max(exp(z)/4, 1/4) and
                # p  = relu(z)*exp(relu(z))/4 == max(z*ex, 0): both identities
                # avoid materializing relu(z).  The /4 (exp bias -ln4) keeps p
                # well inside fp16 range; the num/den ratio is unchanged.
                ex = p1e.tile([K, 2 * FW], F16, tag="ex")
                nc.scalar.activation(out=ex[:], in_=z[:],
                                     func=mybir.ActivationFunctionType.Exp,
                                     bias=nln4_t[:, 0:1])
                nc.vector.tensor_scalar_max(out=ex[:], in0=ex[:], scalar1=0.25)
                pp = p1p.tile([K, 2 * FW], F16, tag="pp")
                nc.vector.tensor_tensor(out=pp[:], in0=z[:], in1=ex[:], op=AX.mult)
                nc.vector.tensor_scalar_max(out=pp[:], in0=pp[:], scalar1=0.0)
                dens = p1w.tile([C, 2 * WN], F32, tag="dens")
                pns = []
                for t in range(2):
                    base = t * FW
                    pd = psD.tile([C, WN], F32, space="PSUM", tag="pd")
                    pn = psN.tile([C, WN], F32, space="PSUM", tag="pn")
                    for n in range(n_fix):
                        sl = slice(base + n * C, base + (n + 1) * C)
                        nc.tensor.matmul(out=pd[:], lhsT=ex[:, sl], rhs=mt[:, sl],
                                         start=(n == 0), stop=(n == n_fix - 1))
                    for n in range(n_fix):
                        sl = slice(base + n * C, base + (n + 1) * C)
                        nc.tensor.matmul(out=pn[:], lhsT=pp[:, sl], rhs=mt[:, sl],
                                         start=(n == 0), stop=(n == n_fix - 1))
                    nc.scalar.activation(out=dens[:, t * WN:(t + 1) * WN], in_=pd[:],
                                         func=mybir.ActivationFunctionType.Identity,
                                         bias=eps_den[0:C, 0:1])
                    pns.append(pn)
                rec = p1w.tile([C, 2 * WN], F32, tag="rec")
                nc.vector.reciprocal(out=rec[:], in_=dens[:])
                agg = p1w.tile([C, 2 * WN], F32, tag="agg")
                for t in range(2):
                    cs = slice(t * WN, (t + 1) * WN)
                    nc.vector.tensor_tensor(out=agg[:, cs], in0=pns[t][:],
                                            in1=rec[:, cs], op=AX.mult)
                npair = 2 * WN if wp < NW // 2 - 1 else WN + LAST_WN
                o = w0 * WN
                ti, to = divmod(o, T)
                nc.vector.tensor_tensor(out=zTs[ti][:, to:to + npair],
                                        in0=agg[:, :npair],
                                        in1=xT[:, o:o + npair], op=AX.add)

        # ================= phase 2: node MLP =============================
        def bn_coeffs(stats_t, g_col, b_col, rows, sfx):
            """AllReduce [CH,2] stats; return per-channel A, B columns."""
            cc_i = dram.tile([CH, 2], F32, tag=f"cci{sfx}")
            cc_o = dram.tile([CH, 2], F32, addr_space="Shared", tag=f"cco{sfx}")
            nc.sync.dma_start(out=cc_i[:], in_=stats_t[:])
            nc.gpsimd.collective_compute(
                "AllReduce", AX.add, ins=[cc_i[:].opt()], outs=[cc_o[:].opt()],
                replica_groups=[list(range(NCORES))])
            g = persist.tile([CH, 2], F32, tag=f"bnred{sfx}")
            nc.sync.dma_start(out=g[:], in_=cc_o[:])
            r = slice(0, rows)
            mean = persist.tile([CH, 1], F32, tag=f"bnm{sfx}")
            nc.vector.tensor_scalar_mul(out=mean[r], in0=g[r, 0:1], scalar1=1.0 / N)
            msq = persist.tile([CH, 1], F32, tag=f"bnq{sfx}")
            nc.vector.tensor_scalar_mul(out=msq[r], in0=g[r, 1:2], scalar1=1.0 / N)
            var = persist.tile([CH, 1], F32, tag=f"bnv{sfx}")
            nc.vector.tensor_tensor(out=var[r], in0=mean[r], in1=mean[r], op=AX.mult)
            nc.vector.tensor_tensor(out=var[r], in0=msq[r], in1=var[r], op=AX.subtract)
            sd = persist.tile([CH, 1], F32, tag=f"bnsd{sfx}")
            nc.scalar.activation(out=sd[r], in_=var[r],
                                 func=mybir.ActivationFunctionType.Sqrt,
                                 bias=eps_t[r, 0:1])
            rsd = persist.tile([CH, 1], F32, tag=f"bnrs{sfx}")
            nc.vector.reciprocal(out=rsd[r], in_=sd[r])
            A = persist.tile([CH, 1], F32, tag=f"bnA{sfx}")
            nc.vector.tensor_tensor(out=A[r], in0=g_col, in1=rsd[r], op=AX.mult)
            B = persist.tile([CH, 1], F32, tag=f"bnB{sfx}")
            nc.vector.tensor_tensor(out=B[r], in0=mean[r], in1=A[r], op=AX.mult)
            nc.vector.tensor_tensor(out=B[r], in0=b_col, in1=B[r], op=AX.subtract)
            return A, B

        def acc_stats(stats_t, hp, tw, rows, sq_pool, st_pool):
            r = slice(0, rows)
            sq = sq_pool.tile([CH, T], F32, tag="sq")
            sqs = st_pool.tile([CH, 1], F32, tag="sqs")
            nc.scalar.activation(out=sq[r, :tw], in_=hp[r, :tw],
                                 func=mybir.ActivationFunctionType.Square,
                                 accum_out=sqs[r])
            s1 = st_pool.tile([CH, 1], F32, tag="s1")
            nc.vector.reduce_sum(out=s1[r], in_=hp[r, :tw], axis=mybir.AxisListType.X)
            nc.vector.tensor_tensor(out=stats_t[r, 0:1], in0=stats_t[r, 0:1],
                                    in1=s1[r], op=AX.add)
            nc.vector.tensor_tensor(out=stats_t[r, 1:2], in0=stats_t[r, 1:2],
                                    in1=sqs[r], op=AX.add)

        bounds = _tile_bounds()
        with (
            tc.tile_pool(name="p2ps", bufs=2, space="PSUM") as p2ps,
            tc.tile_pool(name="p2ps2", bufs=2, space="PSUM") as p2ps2,
            tc.tile_pool(name="p2ps3", bufs=2, space="PSUM") as p2ps3,
            tc.tile_pool(name="p2sq", bufs=2) as p2sq,
            tc.tile_pool(name="p2st", bufs=4) as p2st,
            tc.tile_pool(name="p2r", bufs=2) as p2r,
            tc.tile_pool(name="pst", bufs=2, space="PSUM") as pst,
            tc.tile_pool(name="p2o", bufs=2) as p2o,
        ):
            # -- 2a: h1 = zT'W1 stats ------------------------------------
            for ti, (o, tw) in enumerate(bounds):
                h1p = p2ps.tile([CH, T], F32, space="PSUM", tag="h1p")
                nc.tensor.matmul(out=h1p[:, :tw], lhsT=w1t[:], rhs=zTs[ti][:, :tw],
                                 start=True, stop=True)
                acc_stats(stats[0], h1p, tw, CH, p2sq, p2st)
            A1, B1 = bn_coeffs(stats[0], bnt[:, 0:1], bnt[:, 1:2], CH, 1)

            # -- 2b: r1 = relu(bn1(h1)) persisted; h2 stats ---------------
            for ti, (o, tw) in enumerate(bounds):
                h1p = p2ps.tile([CH, T], F32, space="PSUM", tag="h1p")
                nc.tensor.matmul(out=h1p[:, :tw], lhsT=w1t[:], rhs=zTs[ti][:, :tw],
                                 start=True, stop=True)
                nc.scalar.activation(out=r1T[:, o:o + tw], in_=h1p[:, :tw],
                                     func=mybir.ActivationFunctionType.Relu,
                                     scale=A1[:, 0:1], bias=B1[:, 0:1])
                h2p = p2ps2.tile([C, T], F32, space="PSUM", tag="h2p")
                nc.tensor.matmul(out=h2p[:, :tw], lhsT=w2t[:], rhs=r1T[:, o:o + tw],
                                 start=True, stop=True)
                acc_stats(stats[1], h2p, tw, C, p2sq, p2st)
            A2, B2 = bn_coeffs(stats[1], bnt[0:C, 2:3], bnt[0:C, 3:4], C, 2)

            # -- 2c: u = silu(bn2(h2)) persisted; h3 stats ----------------
            for o, tw in bounds:
                h2p = p2ps2.tile([C, T], F32, space="PSUM", tag="h2p")
                nc.tensor.matmul(out=h2p[:, :tw], lhsT=w2t[:], rhs=r1T[:, o:o + tw],
                                 start=True, stop=True)
                nc.scalar.activation(out=uT[:, o:o + tw], in_=h2p[:, :tw],
                                     func=mybir.ActivationFunctionType.Silu,
                                     scale=A2[0:C, 0:1], bias=B2[0:C, 0:1])
                h3p = p2ps3.tile([C, T], F32, space="PSUM", tag="h3p")
                nc.tensor.matmul(out=h3p[:, :tw], lhsT=wlt[:], rhs=uT[:, o:o + tw],
                                 start=True, stop=True)
                acc_stats(stats[2], h3p, tw, C, p2sq, p2st)
            A3, B3 = bn_coeffs(stats[2], bnt[0:C, 4:5], bnt[0:C, 5:6], C, 3)

            # -- 2d: y = silu(bn3(h3)), transpose, store ------------------
            for o, tw in bounds:
                h3p = p2ps3.tile([C, T], F32, space="PSUM", tag="h3p")
                nc.tensor.matmul(out=h3p[:, :tw], lhsT=wlt[:], rhs=uT[:, o:o + tw],
                                 start=True, stop=True)
                ot = p2o.tile([C, T], F16, tag="ot")
                nc.scalar.activation(out=ot[:, :tw], in_=h3p[:, :tw],
                                     func=mybir.ActivationFunctionType.Silu,
                                     scale=A3[0:C, 0:1], bias=B3[0:C, 0:1])
                nblk = (tw + K - 1) // K
                yt = p2o.tile([K, nblk * C], F32, tag="yt")
                tp = pst.tile([K, 4 * C], F16, space="PSUM", tag="tp")
                for j in range(nblk):
                    bw = min(K, tw - j * K)
                    nc.tensor.transpose(out=tp[:bw, j * C:(j + 1) * C],
                                        in_=ot[:, j * K:j * K + bw],
                                        identity=ident16[0:C, 0:C])
                if tw == T:
                    nc.scalar.activation(out=yt[:], in_=tp[:, :nblk * C],
                                         func=mybir.ActivationFunctionType.Copy)
                else:
                    for j in range(nblk):
                        bw = min(K, tw - j * K)
                        nc.scalar.activation(out=yt[:bw, j * C:(j + 1) * C],
                                             in_=tp[:bw, j * C:(j + 1) * C],
                                             func=mybir.ActivationFunctionType.Copy)
                if tw == T:
                    nc.sync.dma_start(
                        out=y_d[o:o + T, :].rearrange("(j p) c -> p j c", p=K),
                        in_=yt[:].rearrange("p (j c) -> p j c", j=nblk))
                else:
                    for j in range(nblk):
                        bw = min(K, tw - j * K)
                        nc.sync.dma_start(out=y_d[o + j * K: o + j * K + bw, :],
                                          in_=yt[:bw, j * C:(j + 1) * C])
    nc.finalize()
    return nc


def preprocess(x, edge_index, edge_attr, W1, W2, Wl, g_mlp, be_mlp, g1, be1,
               g2, be2):
    src = np.asarray(edge_index[0])
    dst = np.asarray(edge_index[1])
    x = np.asarray(x, dtype=np.float32)
    edge_attr = np.asarray(edge_attr, dtype=np.float32)

    core = dst // NPC
    local = dst - core * NPC
    win = local // WN
    gwin = core * NW + win
    order = np.argsort(gwin, kind="stable")
    gw_s = gwin[order]
    counts = np.bincount(gwin, minlength=NCORES * NW)
    n_fix = int(np.ceil(counts.max() / K))
    starts = np.zeros(NCORES * NW, np.int64)
    np.cumsum(counts[:-1], out=starts[1:])
    rank = np.arange(E, dtype=np.int64) - starts[gw_s]

    e_core = core[order]
    e_w = win[order]
    e_n = (rank // K).astype(np.int64)
    e_p = (rank % K).astype(np.int64)

    ea_w = np.zeros((NCORES, NW, K, n_fix, C), np.float32)
    xg_w = np.zeros((NCORES, NW, K, n_fix, C), np.float32)
    wd_w = np.full((NCORES, NW, K, n_fix), 255.0, np.float32)
    ea_w[e_core, e_w, e_p, e_n] = edge_attr[order]
    xg_w[e_core, e_w, e_p, e_n] = x[src[order]]
    wd_w[e_core, e_w, e_p, e_n] = (local[order] - e_w * WN).astype(np.float32)
    # wdst preload layout: [K, NW*n_fix]
    import ml_dtypes
    mh_w = (wd_w[..., None] == np.arange(C, dtype=np.float32)).astype(ml_dtypes.float8_e4m3)
    mh_w = mh_w.reshape(NCORES, NW, K, n_fix * C)
    ea_w = ea_w.reshape(NCORES, NW, K, n_fix * C)
    xg_w = xg_w.reshape(NCORES, NW, K, n_fix * C)

    bn = np.zeros((CH, 6), np.float32)
    bn[:, 0] = g_mlp
    bn[:, 1] = be_mlp
    bn[:C, 2] = g1
    bn[:C, 3] = be1
    bn[:C, 4] = g2
    bn[:C, 5] = be2

    in_maps = []
    for c in range(NCORES):
        xs = np.ascontiguousarray(x[c * NPC:(c + 1) * NPC].T)
        in_maps.append(dict(
            ea=ea_w[c], xg=xg_w[c], mh=mh_w[c], xt=xs,
            w1=np.asarray(W1, np.float16), w2=np.asarray(W2, np.float16),
            wl=np.asarray(Wl, np.float16), bn=bn,
        ))
    return in_maps, n_fix


_PROG_CACHE = {}


def kernel(x, edge_index, edge_attr, pos, W1, b1, g_mlp, be_mlp, W2, b2,
           g1, be1, Wl, g2, be2):
    # b1/b2 cancel inside the batch norms that directly follow them; pos is
    # unused by the reference.
    in_maps, n_fix = preprocess(x, edge_index, edge_attr, W1, W2, Wl,
                                g_mlp, be_mlp, g1, be1, g2, be2)
    if n_fix not in _PROG_CACHE:
        _PROG_CACHE[n_fix] = build_program(n_fix)
    nc = _PROG_CACHE[n_fix]
    r = run_bass_kernel_spmd(nc, in_maps, list(range(NCORES)))
    return np.concatenate([r.results[c]["y"] for c in range(NCORES)], axis=0)
